# revision 1
# baseline (speedup 1.0000x reference)
"""Trainium2 Bass kernel for nn_CIFAR10Net LIF conv layer.

Reference computation:
  w' = weight-standardized clip(weight) ; conv2d(x, w', pad=1) over (T*B) frames
  LIF scan over T with state (u, sg) [sm/ss are dead state]:
     sg = (sg + I) * (1 - 1/tau_grad);  u = u + sg
     spike = u >= th ; u, sg *= (1 - spike)
Spikes out: [T, B, 128, 32, 32] f32.

Device mapping (per core, B sharded 4/core over 8 cores):
  - partition dim = Cout (128); free = positions (b, h, w)
  - PE: im2col conv (27-row contraction, 4-way row-packed over b) producing
    cg*I into PSUM bank b, then accumulates cg*Id @ sg (fp32) -> psum = sg'_t
  - DVE custom ops:  sg''_t = select(u+sg' < th, sg', 0)
                     u''_t  = select(u+sg' < th, u+sg', 0)
  - spike: ACT Sign(u'') as int8 (spike <=> u''==0), decoded host-side.
"""

import os
import numpy as np

import concourse.bacc as bacc
import concourse.mybir as mybir
import concourse.dve_ops as dve_ops
from concourse.dve_spec import Spec, Src0, Src1, C0, Zero, select, lower
from concourse.dve_uop import DveOpSpec
from concourse.tile import TileContext
from concourse.bass_utils import run_bass_kernel_spmd

# ---------------- constants -------------------------------------------------
T, B, CIN, H, W = 16, 32, 3, 32, 32
COUT, KK = 128, 3
NCORES = 8
BSH = B // NCORES          # 4 batches per core
CG = np.float32(1.0 - 1.0 / 3.5)
NB = 512                   # positions per psum bank (= one batch half)
NHALF = 4 * NB             # 2048 positions per half-step
SPIKE_MODE = os.environ.get("LIF_SPIKE_MODE", "act")  # dve | pool | act
KREPEAT = int(os.environ.get("LIF_KREPEAT", "1"))  # program repetitions (timing)
ABLATE = set(filter(None, os.environ.get("LIF_ABLATE", "").split(",")))  # sim ablations
IDSPLIT = int(os.environ.get("LIF_IDSPLIT", "4"))  # banks 0..IDSPLIT-1 on PE, rest on DVE

# ---------------- custom DVE ops -------------------------------------------
_s = Src0 + Src1


def _register_op(name, spec):
    shas = {}
    for ver in ("v3",):
        uops = lower(spec, ver=ver)
        shas[ver] = DveOpSpec(name=name, opcode=0, uops=uops, rd1_en=True).sha(ver)
    op = dve_ops.DveOp(name, spec, subdim=False, uops_sha=shas)
    for o in dve_ops.OPS:
        if o.name == name:
            return o
    dve_ops.OPS.append(op)
    dve_ops.CUSTOM_DVE_SPECS[name] = spec
    dve_ops._SUB_OPCODE_FOR_NAME[name] = max(dve_ops._SUB_OPCODE_FOR_NAME.values()) + 1
    assert dve_ops._SUB_OPCODE_FOR_NAME[name] < 0x20
    return op


LIF_U = _register_op(
    "LIF_U",
    Spec(
        body=select(_s < C0, _s, Zero),
        reference=lambda in0, in1, s0, s1, imm2: np.where(
            (in0 + in1) < s0, (in0 + in1).astype(np.float32), 0.0
        ).astype(np.float32),
    ),
)
LIF_SG = _register_op(
    "LIF_SG",
    Spec(
        body=select(_s < C0, Src1, Zero),
        reference=lambda in0, in1, s0, s1, imm2: np.where(
            (in0 + in1) < s0, in1, 0.0
        ).astype(np.float32),
    ),
)

# ---------------- device kernel builder -------------------------------------
_NC_CACHE = {}


def _build_nc(krepeat=None):
    krepeat = KREPEAT if krepeat is None else krepeat
    key = (SPIKE_MODE, krepeat, IDSPLIT, tuple(sorted(ABLATE)))
    if key in _NC_CACHE:
        return _NC_CACHE[key]
    f32 = mybir.dt.float32
    nc = bacc.Bacc("TRN2", target_bir_lowering=False)

    xpad = nc.dram_tensor("xpad", [T, COUT, 1156], f32, kind="ExternalInput")
    wmat = nc.dram_tensor("wmat", [COUT, COUT], f32, kind="ExternalInput")
    cgid = nc.dram_tensor("cgid", [COUT, COUT], f32, kind="ExternalInput")
    th = nc.dram_tensor("th", [COUT, 1], f32, kind="ExternalInput")
    spk = nc.dram_tensor(
        "spk", [T, 2, COUT, NHALF],
        mybir.dt.uint8 if SPIKE_MODE != "act" else mybir.dt.int8,
        kind="ExternalOutput",
    )

    with TileContext(nc) as tc, \
         tc.tile_pool(name="const", bufs=1) as cpool, \
         tc.tile_pool(name="state", bufs=1) as spool, \
         tc.tile_pool(name="im", bufs=6) as impool, \
         tc.tile_pool(name="out", bufs=6) as opool, \
         tc.tile_pool(name="ps", bufs=2, space="PSUM") as ppool:

        w_sb = cpool.tile([COUT, COUT], f32, tag="w")
        id_sb = cpool.tile([COUT, COUT], f32, tag="id")
        th_sb = cpool.tile([COUT, 1], f32, tag="th")
        nc.sync.dma_start(w_sb[:], wmat[:])
        nc.sync.dma_start(id_sb[:], cgid[:])
        nc.sync.dma_start(th_sb[:], th[:])

        ubuf = [spool.tile([COUT, 2 * NHALF], f32, tag=f"u{i}", name=f"u{i}") for i in range(2)]
        gbuf = [spool.tile([COUT, 2 * NHALF], f32, tag=f"g{i}", name=f"g{i}") for i in range(2)]
        for _rep in range(krepeat):
          nc.gpsimd.memset(ubuf[0][:], 0.0)
          nc.gpsimd.memset(gbuf[0][:], 0.0)

          for t in range(T):
              ucur, unext = ubuf[t % 2], ubuf[(t + 1) % 2]
              gcur, gnext = gbuf[t % 2], gbuf[(t + 1) % 2]

              im = impool.tile([COUT, 34, 34], f32, tag="im27")
              if "imdma" in ABLATE:
                  nc.vector.memset(im[:, :, :], 0.0)
              else:
                  eng = nc.sync if t % 2 == 0 else nc.scalar
                  eng.dma_start(im[:, :, :], xpad[t, :, :])

              for half in range(2):
                  lo = half * NHALF
                  ps = ppool.tile([COUT, NHALF], f32, tag="ps")
                  for b in range(BSH) if "conv" not in ABLATE else []:
                      nc.tensor.matmul(
                          ps[:, NB * b : NB * (b + 1)],
                          w_sb[32 * b : 32 * b + 27, :],
                          im[32 * b : 32 * b + 27, 16 * half : 16 * half + 16, 0:32],
                          start=True,
                          stop=(t == 0),
                          tile_position=(32 * b, 0),
                          skip_group_check=True,
                      )
                  for b in (range(BSH) if ("idmm" not in ABLATE and t > 0) else []):
                      if b < IDSPLIT:
                          nc.tensor.matmul(
                              ps[:, NB * b : NB * (b + 1)],
                              id_sb[:],
                              gcur[:, lo + NB * b : lo + NB * (b + 1)],
                              start=False,
                              stop=True,
                              tile_position=(0, 0),
                              skip_group_check=True,
                          )
                      else:
                          nc.vector.scalar_tensor_tensor(
                              ps[:, NB * b : NB * (b + 1)],
                              gcur[:, lo + NB * b : lo + NB * (b + 1)],
                              float(CG),
                              ps[:, NB * b : NB * (b + 1)],
                              mybir.AluOpType.mult,
                              mybir.AluOpType.add,
                          )

                  if "dve" in ABLATE:
                      nc.vector.memset(gnext[:, lo : lo + NHALF], 0.0)
                      nc.vector.memset(unext[:, lo : lo + NHALF], 0.0)
                  else:
                      nc.vector._custom_dve(
                          LIF_SG,
                          out=gnext[:, lo : lo + NHALF],
                          in0=ucur[:, lo : lo + NHALF],
                          in1=ps[:],
                          s0=th_sb[:],
                      )
                      nc.vector._custom_dve(
                          LIF_U,
                          out=unext[:, lo : lo + NHALF],
                          in0=ucur[:, lo : lo + NHALF],
                          in1=ps[:],
                          s0=th_sb[:],
                      )

                  if SPIKE_MODE == "act":
                      st = opool.tile([COUT, NHALF], mybir.dt.int8, tag="spk")
                      nc.scalar.activation(
                          st[:], unext[:, lo : lo + NHALF],
                          mybir.ActivationFunctionType.Sign,
                      )
                  elif SPIKE_MODE == "pool":
                      st = opool.tile([COUT, NHALF], mybir.dt.uint8, tag="spk")
                      nc.gpsimd.tensor_scalar(
                          st[:], unext[:, lo : lo + NHALF], 0.0, None,
                          mybir.AluOpType.is_equal,
                      )
                  else:
                      st = opool.tile([COUT, NHALF], mybir.dt.uint8, tag="spk")
                      nc.vector.tensor_scalar(
                          st[:], unext[:, lo : lo + NHALF], 0.0, None,
                          mybir.AluOpType.is_equal,
                      )
                  if "outdma" not in ABLATE:
                      nc.gpsimd.dma_start(spk[t, half, :, :], st[:])

    nc.finalize()
    _NC_CACHE[key] = nc
    return nc


# ---------------- host side --------------------------------------------------
def _prep_weights(weight, norm_weight, norm_bias):
    w = np.clip(weight.astype(np.float32), -4.0, 4.0)
    flat = w.reshape(COUT, -1)
    mean = flat.mean(axis=1, dtype=np.float32)
    var = flat.var(axis=1, ddof=1, dtype=np.float32)
    scale = (norm_weight.reshape(COUT).astype(np.float32)
             / np.sqrt(var + np.float32(1e-5)))
    w_std = (w - mean[:, None, None, None]) * scale[:, None, None, None] \
        + norm_bias.reshape(COUT, 1, 1, 1).astype(np.float32)
    # wmat[32b + 3*(3dy+dx) + c, co] = cg * w_std[co, c, dy, dx]
    wmat = np.zeros((COUT, COUT), np.float32)
    wk = (CG * w_std).transpose(1, 2, 3, 0)  # [c, dy, dx, co]
    for dy in range(3):
        for dx in range(3):
            r = 3 * (3 * dy + dx)
            for b in range(BSH):
                wmat[32 * b + r : 32 * b + r + 3, :] = wk[:, dy, dx, :]
    return wmat


def kernel(x, weight, norm_weight, norm_bias, threshold, _want_trace=False, _krepeat=None):
    x = np.asarray(x, np.float32)
    nc = _build_nc(_krepeat)
    wmat = _prep_weights(np.asarray(weight), np.asarray(norm_weight),
                         np.asarray(norm_bias))
    cgid = (np.eye(COUT) * CG).astype(np.float32)
    th_h = np.asarray(threshold, np.float32).reshape(COUT, 1)

    xp = np.pad(x, [(0, 0), (0, 0), (0, 0), (1, 1), (1, 1)])  # [T,B,C,34,34]
    # x27[t, 32b + 3*(3dy+dx) + c, f] = xpad[t, b, c].flat[34*dy + dx + f]
    xflat = np.pad(xp.reshape(T, B, CIN * 1156), [(0, 0), (0, 0), (0, 128)])
    x27 = np.zeros((T, B, 32, 1156), np.float32)
    for dy in range(3):
        for dx in range(3):
            for c in range(CIN):
                off = c * 1156 + 34 * dy + dx
                x27[:, :, 3 * (3 * dy + dx) + c, :] = xflat[:, :, off : off + 1156]
    in_maps = []
    for core in range(NCORES):
        xs = np.ascontiguousarray(
            x27[:, core * BSH : (core + 1) * BSH].reshape(T, COUT, 1156)
        )
        in_maps.append({"xpad": xs, "wmat": wmat, "cgid": cgid, "th": th_h})

    res = run_bass_kernel_spmd(
        nc, in_maps, core_ids=list(range(NCORES)), trace=_want_trace
    )

    out = np.empty((T, B, COUT, H, W), np.float32)
    for core in range(NCORES):
        s = res.results[core]["spk"]  # [T, 2, 128, 2048]
        if SPIKE_MODE == "act":
            spikes = (s == 0)
        else:
            spikes = (s != 0)
        # [t, half, co, b, hh, w] -> [t, b, co, 16*half+hh, w]
        spikes = spikes.reshape(T, 2, COUT, BSH, 16, W).transpose(0, 3, 2, 1, 4, 5)
        out[:, core * BSH : (core + 1) * BSH] = spikes.reshape(
            T, BSH, COUT, H, W
        ).astype(np.float32)
    if _want_trace:
        kernel.last_result = res
    return out



# revision 10
# speedup vs baseline: 1.3351x; 1.3351x over previous
"""Trainium2 Bass kernel for nn_CIFAR10Net LIF conv layer.

Reference computation:
  w' = weight-standardized clip(weight) ; conv2d(x, w', pad=1) over (T*B) frames
  LIF scan over T with state (u, sg) [sm/ss are dead state]:
     sg = (sg + I) * (1 - 1/tau_grad);  u = u + sg
     spike = u >= th ; u, sg *= (1 - spike)
Spikes out: [T, B, 128, 32, 32] f32.

Device mapping (per core, B sharded 4/core over 8 cores):
  - partition dim = Cout (128); free = positions (b, h, w)
  - PE: im2col conv (27-row contraction, 4-way row-packed over b) producing
    cg*I into PSUM bank b, then accumulates cg*Id @ sg (fp32) -> psum = sg'_t
  - DVE custom ops:  sg''_t = select(u+sg' < th, sg', 0)
                     u''_t  = select(u+sg' < th, u+sg', 0)
  - spike: ACT Sign(u'') as int8 (spike <=> u''==0), decoded host-side.
"""

import os
import numpy as np

import concourse.bacc as bacc
import concourse.mybir as mybir
import concourse.dve_ops as dve_ops
from concourse.dve_spec import Spec, Src0, Src1, C0, Zero, select, lower
from concourse.dve_uop import DveOpSpec
from concourse.tile import TileContext
from concourse.bass_utils import run_bass_kernel_spmd

# ---------------- constants -------------------------------------------------
T, B, CIN, H, W = 16, 32, 3, 32, 32
COUT, KK = 128, 3
NCORES = 8
BSH = B // NCORES          # 4 batches per core
CG = np.float32(1.0 - 1.0 / 3.5)
NB = 512                   # positions per psum bank (= one batch half)
NHALF = 4 * NB             # 2048 positions per half-step
SPIKE_MODE = os.environ.get("LIF_SPIKE_MODE", "act")  # dve | pool | act
KREPEAT = int(os.environ.get("LIF_KREPEAT", "1"))  # program repetitions (timing)
ABLATE = set(filter(None, os.environ.get("LIF_ABLATE", "").split(",")))  # sim ablations
IDSPLIT = int(os.environ.get("LIF_IDSPLIT", "4"))  # banks 0..IDSPLIT-1 on PE, rest on DVE

# ---------------- custom DVE ops -------------------------------------------
_s = Src0 + Src1


def _register_op(name, spec):
    shas = {}
    for ver in ("v3",):
        uops = lower(spec, ver=ver)
        shas[ver] = DveOpSpec(name=name, opcode=0, uops=uops, rd1_en=True).sha(ver)
    op = dve_ops.DveOp(name, spec, subdim=False, uops_sha=shas)
    for o in dve_ops.OPS:
        if o.name == name:
            return o
    dve_ops.OPS.append(op)
    dve_ops.CUSTOM_DVE_SPECS[name] = spec
    dve_ops._SUB_OPCODE_FOR_NAME[name] = max(dve_ops._SUB_OPCODE_FOR_NAME.values()) + 1
    assert dve_ops._SUB_OPCODE_FOR_NAME[name] < 0x20
    return op


LIF_U = _register_op(
    "LIF_U",
    Spec(
        body=select(_s < C0, _s, Zero),
        reference=lambda in0, in1, s0, s1, imm2: np.where(
            (in0 + in1) < s0, (in0 + in1).astype(np.float32), 0.0
        ).astype(np.float32),
    ),
)
LIF_SG = _register_op(
    "LIF_SG",
    Spec(
        body=select(_s < C0, Src1, Zero),
        reference=lambda in0, in1, s0, s1, imm2: np.where(
            (in0 + in1) < s0, in1, 0.0
        ).astype(np.float32),
    ),
)

# ---------------- device kernel builder -------------------------------------
_NC_CACHE = {}


def _build_nc(krepeat=None):
    krepeat = KREPEAT if krepeat is None else krepeat
    key = (SPIKE_MODE, krepeat, IDSPLIT, tuple(sorted(ABLATE)))
    if key in _NC_CACHE:
        return _NC_CACHE[key]
    f32 = mybir.dt.float32
    f32r = mybir.dt.float32r
    nc = bacc.Bacc("TRN2", target_bir_lowering=False)

    xpad = nc.dram_tensor("xpad", [T, COUT, 1156], f32, kind="ExternalInput")
    wmat = nc.dram_tensor("wmat", [COUT, COUT], f32, kind="ExternalInput")
    cgid = nc.dram_tensor("cgid", [COUT, COUT], f32r, kind="ExternalInput")
    th = nc.dram_tensor("th", [COUT, 1], f32, kind="ExternalInput")
    spk = nc.dram_tensor(
        "spk", [T, 2, COUT, NHALF],
        mybir.dt.uint8 if SPIKE_MODE != "act" else mybir.dt.int8,
        kind="ExternalOutput",
    )

    with TileContext(nc) as tc, \
         tc.tile_pool(name="const", bufs=1) as cpool, \
         tc.tile_pool(name="state", bufs=1) as spool, \
         tc.tile_pool(name="im", bufs=6) as impool, \
         tc.tile_pool(name="out", bufs=6) as opool, \
         tc.tile_pool(name="ps", bufs=2, space="PSUM") as ppool:

        w_sb = cpool.tile([COUT, COUT], f32, tag="w")
        id_sb = cpool.tile([COUT, COUT], f32r, tag="id")
        th_sb = cpool.tile([COUT, 1], f32, tag="th")
        nc.sync.dma_start(w_sb[:], wmat[:])
        nc.sync.dma_start(id_sb[:], cgid[:])
        nc.sync.dma_start(th_sb[:], th[:])

        ubuf = [spool.tile([COUT, 2 * NHALF], f32, tag=f"u{i}", name=f"u{i}") for i in range(2)]
        gbuf = [spool.tile([COUT, 2 * NHALF], f32r, tag=f"g{i}", name=f"g{i}") for i in range(2)]
        for _rep in range(krepeat):
          nc.gpsimd.memset(ubuf[0][:], 0.0)
          nc.gpsimd.memset(gbuf[0][:].bitcast(f32), 0.0)

          for t in range(T):
              ucur, unext = ubuf[t % 2], ubuf[(t + 1) % 2]
              gcur, gnext = gbuf[t % 2], gbuf[(t + 1) % 2]

              im = impool.tile([COUT, 34, 34], f32, tag="im27")
              if "imdma" in ABLATE:
                  nc.vector.memset(im[:, :, :], 0.0)
              else:
                  eng = nc.sync if t % 2 == 0 else nc.scalar
                  eng.dma_start(im[:, :, :], xpad[t, :, :])

              for half in range(2):
                  lo = half * NHALF
                  ps = ppool.tile([COUT, NHALF], f32, tag="ps")
                  for b in range(BSH) if "conv" not in ABLATE else []:
                      nc.tensor.matmul(
                          ps[:, NB * b : NB * (b + 1)],
                          w_sb[32 * b : 32 * b + 27, :],
                          im[32 * b : 32 * b + 27, 16 * half : 16 * half + 16, 0:32],
                          start=True,
                          stop=(t == 0),
                          tile_position=(32 * b, 0),
                          skip_group_check=True,
                      )
                  for b in (range(BSH) if ("idmm" not in ABLATE and t > 0) else []):
                      if b < IDSPLIT:
                          nc.tensor.matmul(
                              ps[:, NB * b : NB * (b + 1)],
                              id_sb[:],
                              gcur[:, lo + NB * b : lo + NB * (b + 1)],
                              start=False,
                              stop=True,
                              tile_position=(0, 0),
                              skip_group_check=True,
                          )
                      else:
                          nc.vector.scalar_tensor_tensor(
                              ps[:, NB * b : NB * (b + 1)],
                              gcur[:, lo + NB * b : lo + NB * (b + 1)],
                              float(CG),
                              ps[:, NB * b : NB * (b + 1)],
                              mybir.AluOpType.mult,
                              mybir.AluOpType.add,
                          )

                  if "dve" in ABLATE:
                      nc.vector.memset(gnext[:, lo : lo + NHALF], 0.0)
                      nc.vector.memset(unext[:, lo : lo + NHALF], 0.0)
                  else:
                      nc.vector._custom_dve(
                          LIF_SG,
                          out=gnext[:, lo : lo + NHALF],
                          in0=ucur[:, lo : lo + NHALF],
                          in1=ps[:],
                          s0=th_sb[:],
                      )
                      nc.vector._custom_dve(
                          LIF_U,
                          out=unext[:, lo : lo + NHALF],
                          in0=ucur[:, lo : lo + NHALF],
                          in1=ps[:],
                          s0=th_sb[:],
                      )

                  if SPIKE_MODE == "act":
                      st = opool.tile([COUT, NHALF], mybir.dt.int8, tag="spk")
                      nc.scalar.activation(
                          st[:], unext[:, lo : lo + NHALF],
                          mybir.ActivationFunctionType.Sign,
                      )
                  elif SPIKE_MODE == "pool":
                      st = opool.tile([COUT, NHALF], mybir.dt.uint8, tag="spk")
                      nc.gpsimd.tensor_scalar(
                          st[:], unext[:, lo : lo + NHALF], 0.0, None,
                          mybir.AluOpType.is_equal,
                      )
                  else:
                      st = opool.tile([COUT, NHALF], mybir.dt.uint8, tag="spk")
                      nc.vector.tensor_scalar(
                          st[:], unext[:, lo : lo + NHALF], 0.0, None,
                          mybir.AluOpType.is_equal,
                      )
                  if "outdma" not in ABLATE:
                      nc.gpsimd.dma_start(spk[t, half, :, :], st[:])

    nc.finalize()
    _NC_CACHE[key] = nc
    return nc


# ---------------- host side --------------------------------------------------
def _prep_weights(weight, norm_weight, norm_bias):
    w = np.clip(weight.astype(np.float32), -4.0, 4.0)
    flat = w.reshape(COUT, -1)
    mean = flat.mean(axis=1, dtype=np.float32)
    var = flat.var(axis=1, ddof=1, dtype=np.float32)
    scale = (norm_weight.reshape(COUT).astype(np.float32)
             / np.sqrt(var + np.float32(1e-5)))
    w_std = (w - mean[:, None, None, None]) * scale[:, None, None, None] \
        + norm_bias.reshape(COUT, 1, 1, 1).astype(np.float32)
    # wmat[32b + 3*(3dy+dx) + c, co] = cg * w_std[co, c, dy, dx]
    wmat = np.zeros((COUT, COUT), np.float32)
    wk = (CG * w_std).transpose(1, 2, 3, 0)  # [c, dy, dx, co]
    for dy in range(3):
        for dx in range(3):
            r = 3 * (3 * dy + dx)
            for b in range(BSH):
                wmat[32 * b + r : 32 * b + r + 3, :] = wk[:, dy, dx, :]
    return wmat


def kernel(x, weight, norm_weight, norm_bias, threshold, _want_trace=False, _krepeat=None):
    x = np.asarray(x, np.float32)
    nc = _build_nc(_krepeat)
    wmat = _prep_weights(np.asarray(weight), np.asarray(norm_weight),
                         np.asarray(norm_bias))
    cgid = (np.eye(COUT) * CG).astype(np.float32)
    th_h = np.asarray(threshold, np.float32).reshape(COUT, 1)

    xp = np.pad(x, [(0, 0), (0, 0), (0, 0), (1, 1), (1, 1)])  # [T,B,C,34,34]
    # x27[t, 32b + 3*(3dy+dx) + c, f] = xpad[t, b, c].flat[34*dy + dx + f]
    xflat = np.pad(xp.reshape(T, B, CIN * 1156), [(0, 0), (0, 0), (0, 128)])
    x27 = np.zeros((T, B, 32, 1156), np.float32)
    for dy in range(3):
        for dx in range(3):
            for c in range(CIN):
                off = c * 1156 + 34 * dy + dx
                x27[:, :, 3 * (3 * dy + dx) + c, :] = xflat[:, :, off : off + 1156]
    in_maps = []
    for core in range(NCORES):
        xs = np.ascontiguousarray(
            x27[:, core * BSH : (core + 1) * BSH].reshape(T, COUT, 1156)
        )
        in_maps.append({"xpad": xs, "wmat": wmat, "cgid": cgid, "th": th_h})

    res = run_bass_kernel_spmd(
        nc, in_maps, core_ids=list(range(NCORES)), trace=_want_trace
    )

    out = np.empty((T, B, COUT, H, W), np.float32)
    for core in range(NCORES):
        s = res.results[core]["spk"]  # [T, 2, 128, 2048]
        if SPIKE_MODE == "act":
            spikes = (s == 0)
        else:
            spikes = (s != 0)
        # [t, half, co, b, hh, w] -> [t, b, co, 16*half+hh, w]
        spikes = spikes.reshape(T, 2, COUT, BSH, 16, W).transpose(0, 3, 2, 1, 4, 5)
        out[:, core * BSH : (core + 1) * BSH] = spikes.reshape(
            T, BSH, COUT, H, W
        ).astype(np.float32)
    if _want_trace:
        kernel.last_result = res
    return out



# revision 15
# speedup vs baseline: 1.3550x; 1.0149x over previous
"""Trainium2 Bass kernel for nn_CIFAR10Net LIF conv layer.

Reference computation:
  w' = weight-standardized clip(weight) ; conv2d(x, w', pad=1) over (T*B) frames
  LIF scan over T with state (u, sg) [sm/ss are dead state]:
     sg = (sg + I) * (1 - 1/tau_grad);  u = u + sg
     spike = u >= th ; u, sg *= (1 - spike)
Spikes out: [T, B, 128, 32, 32] f32.

Device mapping (per core, B sharded 4/core over 8 cores):
  - partition dim = Cout (128); free = positions (b, h, w)
  - PE: im2col conv (27-row contraction, 4-way row-packed over b) producing
    cg*I into PSUM bank b, then accumulates cg*Id @ sg (fp32) -> psum = sg'_t
  - DVE custom ops:  sg''_t = select(u+sg' < th, sg', 0)
                     u''_t  = select(u+sg' < th, u+sg', 0)
  - spike: ACT Sign(u'') as int8 (spike <=> u''==0), decoded host-side.
"""

import os
import numpy as np

import concourse.bacc as bacc
import concourse.mybir as mybir
import concourse.dve_ops as dve_ops
from concourse.dve_spec import Spec, Src0, Src1, C0, Zero, select, lower
from concourse.dve_spec import _has_src1
from concourse.dve_uop import DveOpSpec
from concourse.tile import TileContext
from concourse.bass_utils import run_bass_kernel_spmd

# ---------------- constants -------------------------------------------------
T, B, CIN, H, W = 16, 32, 3, 32, 32
COUT, KK = 128, 3
NCORES = 8
BSH = B // NCORES          # 4 batches per core
CG = np.float32(1.0 - 1.0 / 3.5)
NB = 512                   # positions per psum bank (= one batch half)
NHALF = 4 * NB             # 2048 positions per half-step
SPIKE_MODE = os.environ.get("LIF_SPIKE_MODE", "act")  # dve | pool | act
KREPEAT = int(os.environ.get("LIF_KREPEAT", "1"))  # program repetitions (timing)
ABLATE = set(filter(None, os.environ.get("LIF_ABLATE", "").split(",")))  # sim ablations
IDSPLIT = int(os.environ.get("LIF_IDSPLIT", "4"))  # banks 0..IDSPLIT-1 on PE, rest on DVE

# ---------------- custom DVE ops -------------------------------------------
_s = Src0 + Src1


def _register_op(name, spec):
    shas = {}
    for ver in ("v3",):
        uops = lower(spec, ver=ver)
        shas[ver] = DveOpSpec(
            name=name, opcode=0, uops=uops, rd1_en=_has_src1(spec)
        ).sha(ver)
    op = dve_ops.DveOp(name, spec, subdim=False, uops_sha=shas)
    for o in dve_ops.OPS:
        if o.name == name:
            return o
    dve_ops.OPS.append(op)
    dve_ops.CUSTOM_DVE_SPECS[name] = spec
    dve_ops._SUB_OPCODE_FOR_NAME[name] = max(dve_ops._SUB_OPCODE_FOR_NAME.values()) + 1
    assert dve_ops._SUB_OPCODE_FOR_NAME[name] < 0x20
    return op


LIF_U = _register_op(
    "LIF_U",
    Spec(
        body=select(_s < C0, _s, Zero),
        reference=lambda in0, in1, s0, s1, imm2: np.where(
            (in0 + in1) < s0, (in0 + in1).astype(np.float32), 0.0
        ).astype(np.float32),
    ),
)
LIF_SG = _register_op(
    "LIF_SG",
    Spec(
        body=select(_s < C0, Src1, Zero),
        reference=lambda in0, in1, s0, s1, imm2: np.where(
            (in0 + in1) < s0, in1, 0.0
        ).astype(np.float32),
    ),
)
# t=0 state update (u = sg = 0): both new states equal select(ps < th, ps, 0).
LIF_P0 = _register_op(
    "LIF_P0",
    Spec(
        body=select(Src0 < C0, Src0, Zero),
        reference=lambda in0, in1, s0, s1, imm2: np.where(
            in0 < s0, in0, 0.0
        ).astype(np.float32),
    ),
)

# ---------------- device kernel builder -------------------------------------
_NC_CACHE = {}


def _build_nc(krepeat=None):
    krepeat = KREPEAT if krepeat is None else krepeat
    key = (SPIKE_MODE, krepeat, IDSPLIT, tuple(sorted(ABLATE)))
    if key in _NC_CACHE:
        return _NC_CACHE[key]
    f32 = mybir.dt.float32
    f32r = mybir.dt.float32r
    nc = bacc.Bacc("TRN2", target_bir_lowering=False)

    xpad = nc.dram_tensor("xpad", [T, COUT, 1156], f32, kind="ExternalInput")
    wmat = nc.dram_tensor("wmat", [COUT, COUT], f32, kind="ExternalInput")
    cgid = nc.dram_tensor("cgid", [COUT, COUT], f32r, kind="ExternalInput")
    th = nc.dram_tensor("th", [COUT, 1], f32, kind="ExternalInput")
    spk = nc.dram_tensor(
        "spk", [T, 2, COUT, NHALF],
        mybir.dt.uint8 if SPIKE_MODE != "act" else mybir.dt.int8,
        kind="ExternalOutput",
    )

    with TileContext(nc) as tc, \
         tc.tile_pool(name="const", bufs=1) as cpool, \
         tc.tile_pool(name="state", bufs=1) as spool, \
         tc.tile_pool(name="im", bufs=6) as impool, \
         tc.tile_pool(name="out", bufs=6) as opool, \
         tc.tile_pool(name="ps", bufs=2, space="PSUM") as ppool:

        w_sb = cpool.tile([COUT, COUT], f32, tag="w")
        id_sb = cpool.tile([COUT, COUT], f32r, tag="id")
        th_sb = cpool.tile([COUT, 1], f32, tag="th")
        nc.sync.dma_start(w_sb[:], wmat[:])
        nc.sync.dma_start(id_sb[:], cgid[:])
        nc.sync.dma_start(th_sb[:], th[:])

        ubuf = [spool.tile([COUT, 2 * NHALF], f32, tag=f"u{i}", name=f"u{i}") for i in range(2)]
        gbuf = [spool.tile([COUT, 2 * NHALF], f32r, tag=f"g{i}", name=f"g{i}") for i in range(2)]
        for _rep in range(krepeat):
          for t in range(T):
              ucur, unext = ubuf[t % 2], ubuf[(t + 1) % 2]
              gcur, gnext = gbuf[t % 2], gbuf[(t + 1) % 2]

              im = impool.tile([COUT, 34, 34], f32, tag="im27")
              if "imdma" in ABLATE:
                  nc.vector.memset(im[:, :, :], 0.0)
              else:
                  eng = nc.sync if t % 2 == 0 else nc.scalar
                  eng.dma_start(im[:, :, :], xpad[t, :, :])

              for half in range(2):
                  lo = half * NHALF
                  ps = ppool.tile([COUT, NHALF], f32, tag="ps")
                  for b in range(BSH) if "conv" not in ABLATE else []:
                      nc.tensor.matmul(
                          ps[:, NB * b : NB * (b + 1)],
                          w_sb[32 * b : 32 * b + 27, :],
                          im[32 * b : 32 * b + 27, 16 * half : 16 * half + 16, 0:32],
                          start=True,
                          stop=(t == 0),
                          tile_position=(32 * b, 0),
                          skip_group_check=True,
                      )
                  for b in (range(BSH) if ("idmm" not in ABLATE and t > 0) else []):
                      if b < IDSPLIT:
                          nc.tensor.matmul(
                              ps[:, NB * b : NB * (b + 1)],
                              id_sb[:],
                              gcur[:, lo + NB * b : lo + NB * (b + 1)],
                              start=False,
                              stop=True,
                              tile_position=(0, 0),
                              skip_group_check=True,
                          )
                      else:
                          nc.vector.scalar_tensor_tensor(
                              ps[:, NB * b : NB * (b + 1)],
                              gcur[:, lo + NB * b : lo + NB * (b + 1)],
                              float(CG),
                              ps[:, NB * b : NB * (b + 1)],
                              mybir.AluOpType.mult,
                              mybir.AluOpType.add,
                          )

                  if "dve" in ABLATE:
                      nc.vector.memset(gnext[:, lo : lo + NHALF], 0.0)
                      nc.vector.memset(unext[:, lo : lo + NHALF], 0.0)
                  elif t == 0:
                      # u = sg = 0: both updates collapse to sel(ps<th, ps, 0).
                      nc.vector._custom_dve(
                          LIF_P0,
                          out=gnext[:, lo : lo + NHALF],
                          in0=ps[:],
                          s0=th_sb[:],
                      )
                      nc.vector._custom_dve(
                          LIF_P0,
                          out=unext[:, lo : lo + NHALF],
                          in0=ps[:],
                          s0=th_sb[:],
                      )
                  else:
                      if t < T - 1:  # t=T-1 sg state is dead
                          nc.vector._custom_dve(
                              LIF_SG,
                              out=gnext[:, lo : lo + NHALF],
                              in0=ucur[:, lo : lo + NHALF],
                              in1=ps[:],
                              s0=th_sb[:],
                          )
                      nc.vector._custom_dve(
                          LIF_U,
                          out=unext[:, lo : lo + NHALF],
                          in0=ucur[:, lo : lo + NHALF],
                          in1=ps[:],
                          s0=th_sb[:],
                      )

                  if SPIKE_MODE == "act":
                      st = opool.tile([COUT, NHALF], mybir.dt.int8, tag="spk")
                      nc.scalar.activation(
                          st[:], unext[:, lo : lo + NHALF],
                          mybir.ActivationFunctionType.Sign,
                      )
                  elif SPIKE_MODE == "pool":
                      st = opool.tile([COUT, NHALF], mybir.dt.uint8, tag="spk")
                      nc.gpsimd.tensor_scalar(
                          st[:], unext[:, lo : lo + NHALF], 0.0, None,
                          mybir.AluOpType.is_equal,
                      )
                  else:
                      st = opool.tile([COUT, NHALF], mybir.dt.uint8, tag="spk")
                      nc.vector.tensor_scalar(
                          st[:], unext[:, lo : lo + NHALF], 0.0, None,
                          mybir.AluOpType.is_equal,
                      )
                  if "outdma" not in ABLATE:
                      nc.gpsimd.dma_start(spk[t, half, :, :], st[:])

    nc.finalize()
    _NC_CACHE[key] = nc
    return nc


# ---------------- host side --------------------------------------------------
def _prep_weights(weight, norm_weight, norm_bias):
    w = np.clip(weight.astype(np.float32), -4.0, 4.0)
    flat = w.reshape(COUT, -1)
    mean = flat.mean(axis=1, dtype=np.float32)
    var = flat.var(axis=1, ddof=1, dtype=np.float32)
    scale = (norm_weight.reshape(COUT).astype(np.float32)
             / np.sqrt(var + np.float32(1e-5)))
    w_std = (w - mean[:, None, None, None]) * scale[:, None, None, None] \
        + norm_bias.reshape(COUT, 1, 1, 1).astype(np.float32)
    # wmat[32b + 3*(3dy+dx) + c, co] = cg * w_std[co, c, dy, dx]
    wmat = np.zeros((COUT, COUT), np.float32)
    wk = (CG * w_std).transpose(1, 2, 3, 0)  # [c, dy, dx, co]
    for dy in range(3):
        for dx in range(3):
            r = 3 * (3 * dy + dx)
            for b in range(BSH):
                wmat[32 * b + r : 32 * b + r + 3, :] = wk[:, dy, dx, :]
    return wmat


def kernel(x, weight, norm_weight, norm_bias, threshold, _want_trace=False, _krepeat=None):
    x = np.asarray(x, np.float32)
    nc = _build_nc(_krepeat)
    wmat = _prep_weights(np.asarray(weight), np.asarray(norm_weight),
                         np.asarray(norm_bias))
    cgid = (np.eye(COUT) * CG).astype(np.float32)
    th_h = np.asarray(threshold, np.float32).reshape(COUT, 1)

    xp = np.pad(x, [(0, 0), (0, 0), (0, 0), (1, 1), (1, 1)])  # [T,B,C,34,34]
    # x27[t, 32b + 3*(3dy+dx) + c, f] = xpad[t, b, c].flat[34*dy + dx + f]
    xflat = np.pad(xp.reshape(T, B, CIN * 1156), [(0, 0), (0, 0), (0, 128)])
    x27 = np.zeros((T, B, 32, 1156), np.float32)
    for dy in range(3):
        for dx in range(3):
            for c in range(CIN):
                off = c * 1156 + 34 * dy + dx
                x27[:, :, 3 * (3 * dy + dx) + c, :] = xflat[:, :, off : off + 1156]
    in_maps = []
    for core in range(NCORES):
        xs = np.ascontiguousarray(
            x27[:, core * BSH : (core + 1) * BSH].reshape(T, COUT, 1156)
        )
        in_maps.append({"xpad": xs, "wmat": wmat, "cgid": cgid, "th": th_h})

    res = run_bass_kernel_spmd(
        nc, in_maps, core_ids=list(range(NCORES)), trace=_want_trace
    )

    out = np.empty((T, B, COUT, H, W), np.float32)
    for core in range(NCORES):
        s = res.results[core]["spk"]  # [T, 2, 128, 2048]
        if SPIKE_MODE == "act":
            spikes = (s == 0)
        else:
            spikes = (s != 0)
        # [t, half, co, b, hh, w] -> [t, b, co, 16*half+hh, w]
        spikes = spikes.reshape(T, 2, COUT, BSH, 16, W).transpose(0, 3, 2, 1, 4, 5)
        out[:, core * BSH : (core + 1) * BSH] = spikes.reshape(
            T, BSH, COUT, H, W
        ).astype(np.float32)
    if _want_trace:
        kernel.last_result = res
    return out

